# revision 18
# baseline (speedup 1.0000x reference)
"""Distributed Trainium2 Bass kernel for nGPT-style attention (nn_Attention_14448269984093).

Reference math:
  q = rope(x @ wq), k = rope(x @ wk), v = x @ wv          # 16 heads, hd=128
  q = sqk_eff * l2norm(q); k = sqk_eff * l2norm(k)        # sqk_eff = sqk * sqrt(2048)
  out = softmax(sqrt(128) * q k^T) v                      # non-causal
  return out @ wo

Sharding: tensor-parallel over heads across 8 cores (2 heads/core).
wq/wk/wv column-sharded, wo row-sharded, x replicated (bf16, host-transposed
so the kernel never transposes x on-chip).  Output partials are summed with a
chunked bf16 ReduceScatter overlapped with compute.  Each core returns its
[8, 64, 2048] shards; the host reassembles the full [1, 4096, 2048] output.

Scores are bounded (|q|,|k| ~ 1 after the norm, so |score| <= ~13), so the
softmax safely skips max-subtraction: softmax = exp(s) / sum(exp(s)).
"""

import math

import numpy as np
import ml_dtypes

S = 4096
D = 2048
H = 16
HD = 128
N_CORES = 8
H_LOC = H // N_CORES          # 2 heads per core
DH_LOC = H_LOC * HD           # 256
SQRT_HD = math.sqrt(HD)
SQK_SCALE = math.sqrt(D)      # SQK_INIT_VALUE / SQK_INIT_SCALING

NS = S // 128                 # 32 s-tiles
NCT = D // 128                # 16 contraction tiles
QCH = 512                     # query chunk
NQC = S // QCH                # 8 query chunks
NKK = S // 128                # 32 key tiles
SCH = 512                     # phase-1 s-chunk (columns of x^T per DMA)
NSC = S // SCH                # 8 chunks
RS_CHUNK = 512                # rows per ReduceScatter chunk
N_RS = S // RS_CHUNK          # 8 chunks

_CACHE = {}
_TUNE = {"qk_defer": 2, "proj_every": 2}


def _build(rep=1, num_devices=N_CORES, with_rs=True):
    import concourse.bass as bass
    import concourse.mybir as mybir
    import concourse.tile as tile
    from concourse import bacc
    from concourse.masks import make_identity

    DT = mybir.dt
    F32, BF16 = DT.float32, DT.bfloat16
    OP = mybir.AluOpType
    AF = mybir.ActivationFunctionType

    nc = bacc.Bacc("TRN2", target_bir_lowering=False, debug=False,
                   num_devices=num_devices)

    xT_ext = nc.declare_dram_parameter("xT", [D, S], BF16, isOutput=False)
    wqkv_ext = nc.declare_dram_parameter("wqkv", [D, 3 * DH_LOC], BF16, isOutput=False)
    wo_ext = nc.declare_dram_parameter("wo", [DH_LOC, D], BF16, isOutput=False)
    sqk_ext = nc.declare_dram_parameter("sqk", [2 * DH_LOC], F32, isOutput=False)
    cos_ext = nc.declare_dram_parameter("cos2", [S, HD // 2], BF16, isOutput=False)
    sin_ext = nc.declare_dram_parameter("sin2", [S, HD // 2], BF16, isOutput=False)
    out_ext = nc.declare_dram_parameter("out", [N_RS, RS_CHUNK // N_CORES, D], BF16, isOutput=True)

    y_g = [nc.dram_tensor(f"y_partial{g}", [RS_CHUNK, D], BF16) for g in range(N_RS - 1)]
    y7 = [nc.dram_tensor(f"y7_{s}", [RS_CHUNK // 2, D], BF16) for s in range(2)]
    rs_out = [nc.dram_tensor(f"rs_out{g}", [RS_CHUNK // N_CORES, D], BF16) for g in range(N_RS - 1)]
    rs7 = [nc.dram_tensor(f"rs7_{s}", [RS_CHUNK // 2 // N_CORES, D], BF16) for s in range(2)]

    with tile.TileContext(nc) as tc:
        with (
            tc.tile_pool(name="const", bufs=1) as cpool,
            tc.tile_pool(name="big", bufs=1) as big,
            tc.tile_pool(name="work", bufs=3) as work,
            tc.tile_pool(name="xchp", bufs=2) as xchp,
            tc.tile_pool(name="awp", bufs=5) as awp,
            tc.tile_pool(name="dnm", bufs=3) as dnm,
            tc.tile_pool(name="ysbp", bufs=2) as ysbp,
            tc.tile_pool(name="psA", bufs=2, space="PSUM") as psA,
            tc.tile_pool(name="psB", bufs=2, space="PSUM") as psB,
        ):
            # ---------------- phase 0: constants ----------------
            # identity/ones first: they gate the first PE transposes and must
            # not queue behind the big weight DMAs on the gpsimd stream
            ident = cpool.tile([128, 128], BF16, tag="ident")
            make_identity(nc, ident[:])
            ones128 = cpool.tile([128, 1], BF16, tag="ones")
            nc.gpsimd.memset(ones128[:], 1.0)
            ones_sq = cpool.tile([128, 128], BF16, tag="ones_sq")
            nc.gpsimd.memset(ones_sq[:], 1.0)
            xch_pre = xchp.tile([128, NCT, SCH], BF16, tag="xch")
            _xT_r = xT_ext.ap().rearrange("(c p) s -> p c s", p=128)
            for c in range(NCT):
                nc.sync.dma_start(xch_pre[:, c, :], _xT_r[:, c, 0:SCH])
            wqkv_sb = big.tile([128, NCT, 3 * DH_LOC], BF16, tag="wqkv")
            _wq_r = wqkv_ext.ap().rearrange("(c p) n -> p c n", p=128)
            for c in range(NCT):
                nc.sync.dma_start(wqkv_sb[:, c, :], _wq_r[:, c, :])
            wo_sb = big.tile([128, H_LOC, D], BF16, tag="wo")
            nc.sync.dma_start(
                wo_sb[:], wo_ext.ap().rearrange("(h p) n -> p h n", p=128))
            sqk_row = cpool.tile([1, 2 * DH_LOC], F32, tag="sqkr")
            nc.sync.dma_start(sqk_row[:], sqk_ext.ap().unsqueeze(0))
            sqk_bc = cpool.tile([128, 2 * DH_LOC], F32, tag="sqkb")
            nc.gpsimd.partition_broadcast(sqk_bc[:], sqk_row[:])
            cos_sb = big.tile([128, NS, HD // 2], BF16, tag="cos")
            nc.sync.dma_start(
                cos_sb[:], cos_ext.ap().rearrange("(n p) f -> p n f", p=128))
            sin_sb = big.tile([128, NS, HD // 2], BF16, tag="sin")
            nc.sync.dma_start(
                sin_sb[:], sin_ext.ap().rearrange("(n p) f -> p n f", p=128))
            # persistent activations
            qT = big.tile([128, H_LOC, S], BF16, tag="qT")
            kT = big.tile([128, H_LOC, S], BF16, tag="kT")
            v_sb = big.tile([128, NS, DH_LOC], BF16, tag="v")
            o_sb = big.tile([128, H_LOC, S], BF16, tag="o")

            # ---------------- phase 1: qkv + rope + norm + transpose ----------------
            for _rep in range(rep):
              pend = []
              for ch in range(NSC):
                  if _rep == 0 and ch == 0:
                      xch = xch_pre
                  else:
                      xch = xchp.tile([128, NCT, SCH], BF16, tag="xch")
                      for c in range(NCT):
                          nc.sync.dma_start(
                              xch[:, c, :], _xT_r[:, c, ch * SCH:(ch + 1) * SCH])
                  for t in range(SCH // 128):
                      i = ch * (SCH // 128) + t

                      pq = psA.tile([128, 3 * DH_LOC], F32, tag="s")
                      for c in range(NCT):
                          st = (c == 0)
                          sp = (c == NCT - 1)
                          xsl = xch[:, c, t * 128:(t + 1) * 128]
                          nc.tensor.matmul(pq[:, 0:512], xsl,
                                           wqkv_sb[:, c, 0:512], start=st, stop=sp)
                          nc.tensor.matmul(pq[:, 512:768], xsl,
                                           wqkv_sb[:, c, 512:768], start=st, stop=sp)
                      # v: straight copy to bf16 (ACT is idle in phase 1)
                      nc.scalar.activation(v_sb[:, i, :], pq[:, 512:768], AF.Copy)

                      cos_i = cos_sb[:, i, None, :].broadcast_to([128, 4, HD // 2])
                      sin_i = sin_sb[:, i, None, :].broadcast_to([128, 4, HD // 2])
                      qkn = work.tile([128, 2 * DH_LOC], BF16, tag="qkn")
                      nrm = work.tile([128, 2 * H_LOC], F32, tag="nrm")
                      rot = work.tile([128, 2 * DH_LOC], F32, tag="rot")
                      # q and k side by side: strided views over the full 512 cols
                      W = 2 * DH_LOC
                      HB = HD // 2
                      re = pq[:, 0:W:2].rearrange("p (a b) -> p a b", b=HB)
                      im = pq[:, 1:W:2].rearrange("p (a b) -> p a b", b=HB)
                      ore = rot[:, 0:W:2].rearrange("p (a b) -> p a b", b=HB)
                      oim = rot[:, 1:W:2].rearrange("p (a b) -> p a b", b=HB)
                      t1 = work.tile([128, 4, HB], F32, tag="t1")
                      t2 = work.tile([128, 4, HB], F32, tag="t2")
                      # out_re = re*cos - im*sin ; out_im = re*sin + im*cos
                      nc.vector.tensor_tensor(t1[:], im, sin_i, op=OP.mult)
                      nc.vector.tensor_tensor(t2[:], re, cos_i, op=OP.mult)
                      nc.vector.tensor_tensor(ore, t2[:], t1[:], op=OP.subtract)
                      nc.vector.tensor_tensor(t1[:], re, sin_i, op=OP.mult)
                      nc.vector.tensor_tensor(t2[:], im, cos_i, op=OP.mult)
                      nc.vector.tensor_tensor(oim, t1[:], t2[:], op=OP.add)
                      # per-head sum of squares
                      sq = work.tile([128, W], F32, tag="sq")
                      nc.gpsimd.tensor_tensor(sq[:], rot[:], rot[:], op=OP.mult)
                      for tt in range(2):
                          for h in range(H_LOC):
                              nc.vector.tensor_reduce(
                                  nrm[:, 2 * tt + h:2 * tt + h + 1],
                                  sq[:, (tt * H_LOC + h) * HD:(tt * H_LOC + h + 1) * HD],
                                  axis=mybir.AxisListType.X, op=OP.add)
                      # 1/||.||
                      nc.scalar.activation(nrm[:], nrm[:], AF.Sqrt)
                      nc.vector.reciprocal(nrm[:], nrm[:])
                      for tt in range(2):
                          base = tt * DH_LOC
                          for h in range(H_LOC):
                              nc.vector.scalar_tensor_tensor(
                                  out=qkn[:, base + h * HD:base + (h + 1) * HD],
                                  in0=rot[:, base + h * HD:base + (h + 1) * HD],
                                  scalar=nrm[:, 2 * tt + h:2 * tt + h + 1],
                                  in1=sqk_bc[:, base + h * HD:base + (h + 1) * HD],
                                  op0=OP.mult, op1=OP.mult)
                      # transpose + store for a PREVIOUS s-tile (software
                      # pipelining: keeps PE off the critical DVE rope chain)
                      pend.append((i, qkn))
                      if len(pend) > _TUNE["qk_defer"]:
                          pi, pqkn = pend.pop(0)
                          ptq = psB.tile([128, 512], BF16, tag="py")
                          for b in range(4):
                              nc.tensor.transpose(
                                  ptq[:, b * 128:(b + 1) * 128],
                                  pqkn[:, b * HD:(b + 1) * HD], ident[:])
                          for tt, dst in ((0, qT), (1, kT)):
                              if pi >= NS - 6:
                                  nc.vector.tensor_copy(
                                      dst[:, :, pi * 128:(pi + 1) * 128],
                                      ptq[:, tt * 256:(tt + 1) * 256].rearrange(
                                          "p (h d) -> p h d", h=H_LOC))
                              else:
                                  nc.scalar.activation(
                                      dst[:, :, pi * 128:(pi + 1) * 128],
                                      ptq[:, tt * 256:(tt + 1) * 256].rearrange(
                                          "p (h d) -> p h d", h=H_LOC),
                                      AF.Copy)

                # drain pipelined transposes
              for pi, pqkn in pend:
                  ptq = psB.tile([128, 512], BF16, tag="py")
                  for b in range(4):
                      nc.tensor.transpose(
                          ptq[:, b * 128:(b + 1) * 128],
                          pqkn[:, b * HD:(b + 1) * HD], ident[:])
                  for tt, dst in ((0, qT), (1, kT)):
                      nc.vector.tensor_copy(
                          dst[:, :, pi * 128:(pi + 1) * 128],
                          ptq[:, tt * 256:(tt + 1) * 256].rearrange(
                              "p (h d) -> p h d", h=H_LOC))

              # ---------------- phase 2 (attention) + phase 3 (projection + RS) ----------------
              # Projection work for q-chunk qc-1 is emitted one unit per wave
              # while qc's attention runs, filling the PE stalls where AV
              # waits on the exp of the same wave.  The softmax denominator is
              # accumulated entirely on DVE (bf16 pair tree + f32 accumulate)
              # so the PE only streams the scores and AV matmuls.
              proj_pend = []        # deferred projection emitters (closures)
              tail_pend = []        # deferred unit tails (fold/recip/normalize)
              out_pend = []         # rs_out -> out_ext DMAs deferred one RS
              ydma_done = [0] * N_RS

              def emit_proj_unit(n=1):
                  for _ in range(n):
                      if proj_pend:
                          proj_pend.pop(0)()

              def emit_tail_unit():
                  while tail_pend:
                      tail_pend.pop(0)()

              def make_proj(qc):
                  units_p = []
                  for qt in range(4):
                      q0 = qc * QCH + qt * 128
                      g = q0 // RS_CHUNK
                      ysb = ysbp.tile([128, D], BF16, tag="ysb")
                      for n in range(4):
                          def u(qt=qt, n=n, q0=q0, g=g, ysb=ysb, qc=qc):
                              py = psB.tile([128, 512], F32, tag="py")
                              for h in range(H_LOC):
                                  nc.tensor.matmul(
                                      py[:], o_sb[:, h, q0:q0 + 128],
                                      wo_sb[:, h, n * 512:(n + 1) * 512],
                                      start=(h == 0), stop=(h == H_LOC - 1))
                              if qc == NQC - 1 and n % 2 == 0:
                                  # final drain: ACT is idle after the last exp,
                                  # and alternating engines keeps the 2-slot py
                                  # ring from serializing on DVE
                                  nc.scalar.activation(
                                      ysb[:, n * 512:(n + 1) * 512], py[:], AF.Copy)
                              else:
                                  nc.vector.tensor_copy(ysb[:, n * 512:(n + 1) * 512], py[:])
                              if n == 3:
                                  if g < N_RS - 1:
                                      nc.sync.dma_start(
                                          y_g[g][q0 - g * RS_CHUNK:q0 - g * RS_CHUNK + 128, :],
                                          ysb[:])
                                  else:
                                      sub = (q0 - g * RS_CHUNK) // (RS_CHUNK // 2)
                                      r0 = q0 - g * RS_CHUNK - sub * (RS_CHUNK // 2)
                                      nc.sync.dma_start(y7[sub][r0:r0 + 128, :], ysb[:])
                                  ydma_done[g] += 1
                                  if with_rs and g < N_RS - 1 and ydma_done[g] == RS_CHUNK // 128:
                                      nc.gpsimd.collective_compute(
                                          "ReduceScatter", OP.add,
                                          replica_groups=[list(range(N_CORES))],
                                          ins=[y_g[g].ap().opt()],
                                          outs=[rs_out[g].ap().opt()],
                                      )
                                      # defer the rs_out -> out_ext DMA until the
                                      # next RS fires: issuing it now would park
                                      # the Sync queue on this RS's completion
                                      # and stall all later y-row DMAs behind it
                                      if out_pend:
                                          out_pend.pop(0)()
                                      out_pend.append(
                                          lambda g=g: nc.sync.dma_start(
                                              out_ext[g], rs_out[g][:]))
                                  elif with_rs and g == N_RS - 1 and ydma_done[g] in (2, 4):
                                      sub = ydma_done[g] // 2 - 1
                                      w7 = RS_CHUNK // 2 // N_CORES
                                      nc.gpsimd.collective_compute(
                                          "ReduceScatter", OP.add,
                                          replica_groups=[list(range(N_CORES))],
                                          ins=[y7[sub].ap().opt()],
                                          outs=[rs7[sub].ap().opt()],
                                      )
                                      if out_pend:
                                          out_pend.pop(0)()
                                      out_pend.append(
                                          lambda g=g, sub=sub, w7=w7:
                                          nc.sync.dma_start(
                                              out_ext[g][sub * w7:(sub + 1) * w7, :],
                                              rs7[sub][:]))
                          units_p.append(u)
                  return units_p

              def make_tail(qc, h, po, dacc):
                  def tail():
                      # partition-reduce the f32 denominator via one all-ones
                      # matmul: the [128,128] ones stationary replicates the
                      # column sums onto every partition, so the broadcast is
                      # free and the gpsimd queue (which also carries the
                      # ReduceScatter triggers) stays off this critical path.
                      dacc_bf = work.tile([128, QCH], BF16, tag="daccb")
                      nc.vector.tensor_copy(dacc_bf[:], dacc[:])
                      pd = psB.tile([128, QCH], F32, tag="py")
                      nc.tensor.matmul(pd[:], ones_sq[:], dacc_bf[:],
                                       start=True, stop=True)
                      rb = work.tile([128, QCH], F32, tag="rb")
                      nc.vector.reciprocal(rb[:], pd[:])
                      nc.vector.tensor_tensor(
                          o_sb[:, h, qc * QCH:(qc + 1) * QCH], po[:], rb[:], op=OP.mult)
                  return tail

              # global software pipeline over (unit, kw): scores/exp stream
              # continuously across unit boundaries; AV + denominator trail by
              # DEFER waves; unit tails and the output projection interleave.
              units = [(qc, h) for qc in range(NQC) for h in range(H_LOC)]
              ustate = {}

              def get_state(u):
                  if u not in ustate:
                      po_u = psB.tile([128, QCH], F32, tag="po", name=f"po{u}")
                      dacc_u = work.tile([128, QCH], F32, tag="dacc", name=f"dacc{u}")
                      ustate[u] = {"po": po_u, "dacc": dacc_u, "pair": [],
                                   "first": True}
                  return ustate[u]

              def av_group(u, kw, aw):
                  qc, h = units[u]
                  st_u = get_state(u)
                  po, dacc, pair = st_u["po"], st_u["dacc"], st_u["pair"]
                  for j in range(2):
                      kk = 2 * kw + j
                      nc.tensor.matmul(
                          po[:], v_sb[:, kk, h * HD:(h + 1) * HD],
                          aw[:, j * 512:(j + 1) * 512],
                          start=(kk == 0), stop=(kk == NKK - 1))
                  # denominator: level-1 bf16 pair add (two key tiles)
                  p1 = dnm.tile([128, QCH], BF16, tag="p1")
                  nc.vector.tensor_tensor(
                      p1[:], aw[:, 0:512], aw[:, 512:1024], op=OP.add)
                  pair.append(p1)
                  if len(pair) == 2:
                      a, b = pair[0], pair[1]
                      pair.clear()
                      if st_u["first"]:
                          st_u["first"] = False
                          nc.vector.tensor_tensor(dacc[:], a[:], b[:], op=OP.add)
                      else:
                          p2 = dnm.tile([128, QCH], BF16, tag="p2")
                          nc.vector.tensor_tensor(p2[:], a[:], b[:], op=OP.add)
                          nc.vector.tensor_tensor(dacc[:], dacc[:], p2[:], op=OP.add)
                  if kw == NKK // 2 - 1:
                      tail_pend.append(make_tail(qc, h, po, dacc))
                      if h == H_LOC - 1:
                          proj_pend.extend(make_proj(qc))

              aw_q = []
              DEFER = 3
              for u, (qc, h) in enumerate(units):
                  for kw in range(NKK // 2):
                      ps = psA.tile([128, 1024], F32, tag="s")
                      for j in range(2):
                          kk = 2 * kw + j
                          nc.tensor.matmul(
                              ps[:, j * 512:(j + 1) * 512],
                              kT[:, h, kk * 128:(kk + 1) * 128],
                              qT[:, h, qc * QCH:(qc + 1) * QCH],
                              start=True, stop=True)
                      aw = awp.tile([128, 1024], BF16, tag="aw")
                      nc.scalar.activation(aw[:], ps[:], AF.Exp, scale=SQRT_HD)
                      aw_q.append((u, kw, aw))
                      if len(aw_q) > DEFER:
                          if kw >= 6:
                              emit_tail_unit()
                          av_group(*aw_q.pop(0))
                      if kw >= 7 and kw % 2 == 1:
                          emit_proj_unit(2 if kw >= 11 else 1)
              while aw_q:
                  av_group(*aw_q.pop(0))
              emit_tail_unit()
              _flush_out = out_pend

              while proj_pend:
                  emit_proj_unit()
              while out_pend:
                  out_pend.pop(0)()

    nc.compile()
    return nc


def _get_nc():
    if "nc" not in _CACHE:
        _CACHE["nc"] = _build()
    return _CACHE["nc"]


def make_in_maps(x, freqs_cos, freqs_sin, wq, wk, wv, wo, sqk):
    bf16 = ml_dtypes.bfloat16
    xT = np.ascontiguousarray(
        np.asarray(x, np.float32).reshape(S, D).T).astype(bf16)
    cosk = np.ascontiguousarray(np.asarray(freqs_cos, np.float32)).astype(bf16)
    sink = np.ascontiguousarray(np.asarray(freqs_sin, np.float32)).astype(bf16)
    wq = np.asarray(wq, np.float32)
    wk = np.asarray(wk, np.float32)
    wv = np.asarray(wv, np.float32)
    wo = np.asarray(wo, np.float32)
    sqk_eff = (np.asarray(sqk, np.float32) * SQK_SCALE).astype(np.float32)
    in_maps = []
    for i in range(N_CORES):
        cols = slice(i * DH_LOC, (i + 1) * DH_LOC)
        wqkv = np.concatenate([wq[:, cols], wk[:, cols], wv[:, cols]], axis=1)
        in_maps.append({
            "xT": xT,
            "wqkv": np.ascontiguousarray(wqkv).astype(bf16),
            "wo": np.ascontiguousarray(wo[cols, :]).astype(bf16),
            "sqk": np.ascontiguousarray(
                np.concatenate([sqk_eff[cols]] * 2)),
            "cos2": cosk,
            "sin2": sink,
        })
    return in_maps


def assemble(results):
    y = np.empty((S, D), np.float32)
    w = RS_CHUNK // N_CORES
    w7 = RS_CHUNK // 2 // N_CORES
    for i in range(N_CORES):
        o = np.asarray(results[i]["out"]).astype(np.float32)
        for g in range(N_RS - 1):
            y[g * RS_CHUNK + i * w:g * RS_CHUNK + (i + 1) * w, :] = o[g]
        g = N_RS - 1
        base = g * RS_CHUNK
        for sub in range(2):
            r0 = base + sub * (RS_CHUNK // 2) + i * w7
            y[r0:r0 + w7, :] = o[g][sub * w7:(sub + 1) * w7]
    return y.reshape(1, S, D)


def kernel(**inputs):
    from concourse import bass_utils

    nc = _get_nc()
    in_maps = make_in_maps(**inputs)
    res = bass_utils.run_bass_kernel_spmd(nc, in_maps, core_ids=list(range(N_CORES)))
    return assemble(res.results)


# revision 19
# speedup vs baseline: 1.0202x; 1.0202x over previous
"""Distributed Trainium2 Bass kernel for nGPT-style attention (nn_Attention_14448269984093).

Reference math:
  q = rope(x @ wq), k = rope(x @ wk), v = x @ wv          # 16 heads, hd=128
  q = sqk_eff * l2norm(q); k = sqk_eff * l2norm(k)        # sqk_eff = sqk * sqrt(2048)
  out = softmax(sqrt(128) * q k^T) v                      # non-causal
  return out @ wo

Sharding: tensor-parallel over heads across 8 cores (2 heads/core).
wq/wk/wv column-sharded, wo row-sharded, x replicated (bf16, host-transposed
so the kernel never transposes x on-chip).  Output partials are summed with a
chunked bf16 ReduceScatter overlapped with compute.  Each core returns its
[8, 64, 2048] shards; the host reassembles the full [1, 4096, 2048] output.

Scores are bounded (|q|,|k| ~ 1 after the norm, so |score| <= ~13), so the
softmax safely skips max-subtraction: softmax = exp(s) / sum(exp(s)).
"""

import math

import numpy as np
import ml_dtypes

S = 4096
D = 2048
H = 16
HD = 128
N_CORES = 8
H_LOC = H // N_CORES          # 2 heads per core
DH_LOC = H_LOC * HD           # 256
SQRT_HD = math.sqrt(HD)
SQK_SCALE = math.sqrt(D)      # SQK_INIT_VALUE / SQK_INIT_SCALING

NS = S // 128                 # 32 s-tiles
NCT = D // 128                # 16 contraction tiles
QCH = 512                     # query chunk
NQC = S // QCH                # 8 query chunks
NKK = S // 128                # 32 key tiles
SCH = 512                     # phase-1 s-chunk (columns of x^T per DMA)
NSC = S // SCH                # 8 chunks
RS_CHUNK = 512                # rows per ReduceScatter chunk
N_RS = S // RS_CHUNK          # 8 chunks

_CACHE = {}
_TUNE = {"qk_defer": 2, "proj_every": 2}


def _build(rep=1, num_devices=N_CORES, with_rs=True):
    import concourse.bass as bass
    import concourse.mybir as mybir
    import concourse.tile as tile
    from concourse import bacc
    from concourse.masks import make_identity

    DT = mybir.dt
    F32, BF16 = DT.float32, DT.bfloat16
    OP = mybir.AluOpType
    AF = mybir.ActivationFunctionType

    nc = bacc.Bacc("TRN2", target_bir_lowering=False, debug=False,
                   num_devices=num_devices)

    xT_ext = nc.declare_dram_parameter("xT", [D, S], BF16, isOutput=False)
    wqkv_ext = nc.declare_dram_parameter("wqkv", [D, 3 * DH_LOC], BF16, isOutput=False)
    wo_ext = nc.declare_dram_parameter("wo", [DH_LOC, D], BF16, isOutput=False)
    sqk_ext = nc.declare_dram_parameter("sqk", [2 * DH_LOC], F32, isOutput=False)
    cos_ext = nc.declare_dram_parameter("cos2", [S, HD // 2], BF16, isOutput=False)
    sin_ext = nc.declare_dram_parameter("sin2", [S, HD // 2], BF16, isOutput=False)
    out_ext = nc.declare_dram_parameter("out", [N_RS, RS_CHUNK // N_CORES, D], BF16, isOutput=True)

    y_g = [nc.dram_tensor(f"y_partial{g}", [RS_CHUNK, D], BF16) for g in range(N_RS - 1)]
    y7 = [nc.dram_tensor(f"y7_{s}", [RS_CHUNK // 2, D], BF16) for s in range(2)]
    rs_out = [nc.dram_tensor(f"rs_out{g}", [RS_CHUNK // N_CORES, D], BF16) for g in range(N_RS - 1)]
    rs7 = [nc.dram_tensor(f"rs7_{s}", [RS_CHUNK // 2 // N_CORES, D], BF16) for s in range(2)]

    with tile.TileContext(nc) as tc:
        with (
            tc.tile_pool(name="const", bufs=1) as cpool,
            tc.tile_pool(name="big", bufs=1) as big,
            tc.tile_pool(name="work", bufs=3) as work,
            tc.tile_pool(name="xchp", bufs=2) as xchp,
            tc.tile_pool(name="awp", bufs=6) as awp,
            tc.tile_pool(name="dnm", bufs=3) as dnm,
            tc.tile_pool(name="ysbp", bufs=2) as ysbp,
            tc.tile_pool(name="psA", bufs=2, space="PSUM") as psA,
            tc.tile_pool(name="psB", bufs=2, space="PSUM") as psB,
        ):
            # ---------------- phase 0: constants ----------------
            # identity/ones first: they gate the first PE transposes and must
            # not queue behind the big weight DMAs on the gpsimd stream
            ident = cpool.tile([128, 128], BF16, tag="ident")
            make_identity(nc, ident[:])
            ones128 = cpool.tile([128, 1], BF16, tag="ones")
            nc.gpsimd.memset(ones128[:], 1.0)
            ones_sq = cpool.tile([128, 128], BF16, tag="ones_sq")
            nc.gpsimd.memset(ones_sq[:], 1.0)
            xch_pre = xchp.tile([128, NCT, SCH], BF16, tag="xch")
            _xT_r = xT_ext.ap().rearrange("(c p) s -> p c s", p=128)
            for c in range(NCT):
                nc.sync.dma_start(xch_pre[:, c, :], _xT_r[:, c, 0:SCH])
            wqkv_sb = big.tile([128, NCT, 3 * DH_LOC], BF16, tag="wqkv")
            _wq_r = wqkv_ext.ap().rearrange("(c p) n -> p c n", p=128)
            for c in range(NCT):
                nc.sync.dma_start(wqkv_sb[:, c, :], _wq_r[:, c, :])
            wo_sb = big.tile([128, H_LOC, D], BF16, tag="wo")
            nc.sync.dma_start(
                wo_sb[:], wo_ext.ap().rearrange("(h p) n -> p h n", p=128))
            sqk_row = cpool.tile([1, 2 * DH_LOC], F32, tag="sqkr")
            nc.sync.dma_start(sqk_row[:], sqk_ext.ap().unsqueeze(0))
            sqk_bc = cpool.tile([128, 2 * DH_LOC], F32, tag="sqkb")
            nc.gpsimd.partition_broadcast(sqk_bc[:], sqk_row[:])
            cos_sb = big.tile([128, NS, HD // 2], BF16, tag="cos")
            nc.sync.dma_start(
                cos_sb[:], cos_ext.ap().rearrange("(n p) f -> p n f", p=128))
            sin_sb = big.tile([128, NS, HD // 2], BF16, tag="sin")
            nc.sync.dma_start(
                sin_sb[:], sin_ext.ap().rearrange("(n p) f -> p n f", p=128))
            # persistent activations
            qT = big.tile([128, H_LOC, S], BF16, tag="qT")
            kT = big.tile([128, H_LOC, S], BF16, tag="kT")
            v_sb = big.tile([128, NS, DH_LOC], BF16, tag="v")
            o_sb = big.tile([128, H_LOC, S], BF16, tag="o")

            # ---------------- phase 1: qkv + rope + norm + transpose ----------------
            for _rep in range(rep):
              pend = []
              for ch in range(NSC):
                  if _rep == 0 and ch == 0:
                      xch = xch_pre
                  else:
                      xch = xchp.tile([128, NCT, SCH], BF16, tag="xch")
                      for c in range(NCT):
                          nc.sync.dma_start(
                              xch[:, c, :], _xT_r[:, c, ch * SCH:(ch + 1) * SCH])
                  for t in range(SCH // 128):
                      i = ch * (SCH // 128) + t

                      pq = psA.tile([128, 3 * DH_LOC], F32, tag="s")
                      for c in range(NCT):
                          st = (c == 0)
                          sp = (c == NCT - 1)
                          xsl = xch[:, c, t * 128:(t + 1) * 128]
                          nc.tensor.matmul(pq[:, 0:512], xsl,
                                           wqkv_sb[:, c, 0:512], start=st, stop=sp)
                          nc.tensor.matmul(pq[:, 512:768], xsl,
                                           wqkv_sb[:, c, 512:768], start=st, stop=sp)
                      # v: straight copy to bf16 (ACT is idle in phase 1)
                      nc.scalar.activation(v_sb[:, i, :], pq[:, 512:768], AF.Copy)

                      cos_i = cos_sb[:, i, None, :].broadcast_to([128, 4, HD // 2])
                      sin_i = sin_sb[:, i, None, :].broadcast_to([128, 4, HD // 2])
                      qkn = work.tile([128, 2 * DH_LOC], BF16, tag="qkn")
                      nrm = work.tile([128, 2 * H_LOC], F32, tag="nrm")
                      rot = work.tile([128, 2 * DH_LOC], F32, tag="rot")
                      # q and k side by side: strided views over the full 512 cols
                      W = 2 * DH_LOC
                      HB = HD // 2
                      re = pq[:, 0:W:2].rearrange("p (a b) -> p a b", b=HB)
                      im = pq[:, 1:W:2].rearrange("p (a b) -> p a b", b=HB)
                      ore = rot[:, 0:W:2].rearrange("p (a b) -> p a b", b=HB)
                      oim = rot[:, 1:W:2].rearrange("p (a b) -> p a b", b=HB)
                      t1 = work.tile([128, 4, HB], F32, tag="t1")
                      t2 = work.tile([128, 4, HB], F32, tag="t2")
                      # out_re = re*cos - im*sin ; out_im = re*sin + im*cos
                      nc.vector.tensor_tensor(t1[:], im, sin_i, op=OP.mult)
                      nc.vector.tensor_tensor(t2[:], re, cos_i, op=OP.mult)
                      nc.vector.tensor_tensor(ore, t2[:], t1[:], op=OP.subtract)
                      nc.vector.tensor_tensor(t1[:], re, sin_i, op=OP.mult)
                      nc.vector.tensor_tensor(t2[:], im, cos_i, op=OP.mult)
                      nc.vector.tensor_tensor(oim, t1[:], t2[:], op=OP.add)
                      # per-head sum of squares
                      sq = work.tile([128, W], F32, tag="sq")
                      nc.gpsimd.tensor_tensor(sq[:], rot[:], rot[:], op=OP.mult)
                      for tt in range(2):
                          for h in range(H_LOC):
                              nc.vector.tensor_reduce(
                                  nrm[:, 2 * tt + h:2 * tt + h + 1],
                                  sq[:, (tt * H_LOC + h) * HD:(tt * H_LOC + h + 1) * HD],
                                  axis=mybir.AxisListType.X, op=OP.add)
                      # 1/||.||
                      nc.scalar.activation(nrm[:], nrm[:], AF.Sqrt)
                      nc.vector.reciprocal(nrm[:], nrm[:])
                      for tt in range(2):
                          base = tt * DH_LOC
                          for h in range(H_LOC):
                              nc.vector.scalar_tensor_tensor(
                                  out=qkn[:, base + h * HD:base + (h + 1) * HD],
                                  in0=rot[:, base + h * HD:base + (h + 1) * HD],
                                  scalar=nrm[:, 2 * tt + h:2 * tt + h + 1],
                                  in1=sqk_bc[:, base + h * HD:base + (h + 1) * HD],
                                  op0=OP.mult, op1=OP.mult)
                      # transpose + store for a PREVIOUS s-tile (software
                      # pipelining: keeps PE off the critical DVE rope chain)
                      pend.append((i, qkn))
                      if len(pend) > _TUNE["qk_defer"]:
                          pi, pqkn = pend.pop(0)
                          ptq = psB.tile([128, 512], BF16, tag="py")
                          for b in range(4):
                              nc.tensor.transpose(
                                  ptq[:, b * 128:(b + 1) * 128],
                                  pqkn[:, b * HD:(b + 1) * HD], ident[:])
                          for tt, dst in ((0, qT), (1, kT)):
                              if pi >= NS - 6:
                                  nc.vector.tensor_copy(
                                      dst[:, :, pi * 128:(pi + 1) * 128],
                                      ptq[:, tt * 256:(tt + 1) * 256].rearrange(
                                          "p (h d) -> p h d", h=H_LOC))
                              else:
                                  nc.scalar.activation(
                                      dst[:, :, pi * 128:(pi + 1) * 128],
                                      ptq[:, tt * 256:(tt + 1) * 256].rearrange(
                                          "p (h d) -> p h d", h=H_LOC),
                                      AF.Copy)

                # drain pipelined transposes
              for pi, pqkn in pend:
                  ptq = psB.tile([128, 512], BF16, tag="py")
                  for b in range(4):
                      nc.tensor.transpose(
                          ptq[:, b * 128:(b + 1) * 128],
                          pqkn[:, b * HD:(b + 1) * HD], ident[:])
                  for tt, dst in ((0, qT), (1, kT)):
                      nc.vector.tensor_copy(
                          dst[:, :, pi * 128:(pi + 1) * 128],
                          ptq[:, tt * 256:(tt + 1) * 256].rearrange(
                              "p (h d) -> p h d", h=H_LOC))

              # ---------------- phase 2 (attention) + phase 3 (projection + RS) ----------------
              # Projection work for q-chunk qc-1 is emitted one unit per wave
              # while qc's attention runs, filling the PE stalls where AV
              # waits on the exp of the same wave.  The softmax denominator is
              # accumulated entirely on DVE (bf16 pair tree + f32 accumulate)
              # so the PE only streams the scores and AV matmuls.
              proj_pend = []        # deferred projection emitters (closures)
              tail_pend = []        # deferred unit tails (fold/recip/normalize)
              out_pend = []         # rs_out -> out_ext DMAs deferred one RS
              ydma_done = [0] * N_RS

              def emit_proj_unit(n=1):
                  for _ in range(n):
                      if proj_pend:
                          proj_pend.pop(0)()

              def emit_tail_unit():
                  while tail_pend:
                      tail_pend.pop(0)()

              def make_proj(qc):
                  units_p = []
                  for qt in range(4):
                      q0 = qc * QCH + qt * 128
                      g = q0 // RS_CHUNK
                      ysb = ysbp.tile([128, D], BF16, tag="ysb")
                      for n in range(4):
                          def u(qt=qt, n=n, q0=q0, g=g, ysb=ysb, qc=qc):
                              py = psB.tile([128, 512], F32, tag="py")
                              for h in range(H_LOC):
                                  nc.tensor.matmul(
                                      py[:], o_sb[:, h, q0:q0 + 128],
                                      wo_sb[:, h, n * 512:(n + 1) * 512],
                                      start=(h == 0), stop=(h == H_LOC - 1))
                              if qc == NQC - 1 and n % 2 == 0:
                                  # final drain: ACT is idle after the last exp,
                                  # and alternating engines keeps the 2-slot py
                                  # ring from serializing on DVE
                                  nc.scalar.activation(
                                      ysb[:, n * 512:(n + 1) * 512], py[:], AF.Copy)
                              else:
                                  nc.vector.tensor_copy(ysb[:, n * 512:(n + 1) * 512], py[:])
                              if n == 3:
                                  if g < N_RS - 1:
                                      nc.sync.dma_start(
                                          y_g[g][q0 - g * RS_CHUNK:q0 - g * RS_CHUNK + 128, :],
                                          ysb[:])
                                  else:
                                      sub = (q0 - g * RS_CHUNK) // (RS_CHUNK // 2)
                                      r0 = q0 - g * RS_CHUNK - sub * (RS_CHUNK // 2)
                                      nc.sync.dma_start(y7[sub][r0:r0 + 128, :], ysb[:])
                                  ydma_done[g] += 1
                                  if with_rs and g < N_RS - 1 and ydma_done[g] == RS_CHUNK // 128:
                                      nc.gpsimd.collective_compute(
                                          "ReduceScatter", OP.add,
                                          replica_groups=[list(range(N_CORES))],
                                          ins=[y_g[g].ap().opt()],
                                          outs=[rs_out[g].ap().opt()],
                                      )
                                      # defer the rs_out -> out_ext DMA until the
                                      # next RS fires: issuing it now would park
                                      # the Sync queue on this RS's completion
                                      # and stall all later y-row DMAs behind it
                                      if out_pend:
                                          out_pend.pop(0)()
                                      out_pend.append(
                                          lambda g=g: nc.sync.dma_start(
                                              out_ext[g], rs_out[g][:]))
                                  elif with_rs and g == N_RS - 1 and ydma_done[g] in (2, 4):
                                      sub = ydma_done[g] // 2 - 1
                                      w7 = RS_CHUNK // 2 // N_CORES
                                      nc.gpsimd.collective_compute(
                                          "ReduceScatter", OP.add,
                                          replica_groups=[list(range(N_CORES))],
                                          ins=[y7[sub].ap().opt()],
                                          outs=[rs7[sub].ap().opt()],
                                      )
                                      if out_pend:
                                          out_pend.pop(0)()
                                      out_pend.append(
                                          lambda g=g, sub=sub, w7=w7:
                                          nc.sync.dma_start(
                                              out_ext[g][sub * w7:(sub + 1) * w7, :],
                                              rs7[sub][:]))
                          units_p.append(u)
                  return units_p

              def make_tail(qc, h, po, dacc):
                  def tail():
                      # partition-reduce the f32 denominator via one all-ones
                      # matmul: the [128,128] ones stationary replicates the
                      # column sums onto every partition, so the broadcast is
                      # free and the gpsimd queue (which also carries the
                      # ReduceScatter triggers) stays off this critical path.
                      dacc_bf = work.tile([128, QCH], BF16, tag="daccb")
                      nc.vector.tensor_copy(dacc_bf[:], dacc[:])
                      pd = psB.tile([128, QCH], F32, tag="py")
                      nc.tensor.matmul(pd[:], ones_sq[:], dacc_bf[:],
                                       start=True, stop=True)
                      rb = work.tile([128, QCH], F32, tag="rb")
                      nc.vector.reciprocal(rb[:], pd[:])
                      nc.vector.tensor_tensor(
                          o_sb[:, h, qc * QCH:(qc + 1) * QCH], po[:], rb[:], op=OP.mult)
                  return tail

              # global software pipeline over (unit, kw): scores/exp stream
              # continuously across unit boundaries; AV + denominator trail by
              # DEFER waves; unit tails and the output projection interleave.
              units = [(qc, h) for qc in range(NQC) for h in range(H_LOC)]
              ustate = {}

              def get_state(u):
                  if u not in ustate:
                      po_u = psB.tile([128, QCH], F32, tag="po", name=f"po{u}")
                      dacc_u = work.tile([128, QCH], F32, tag="dacc", name=f"dacc{u}")
                      ustate[u] = {"po": po_u, "dacc": dacc_u, "pair": [],
                                   "first": True}
                  return ustate[u]

              def av_group(u, kw, aw):
                  qc, h = units[u]
                  st_u = get_state(u)
                  po, dacc, pair = st_u["po"], st_u["dacc"], st_u["pair"]
                  for j in range(2):
                      kk = 2 * kw + j
                      nc.tensor.matmul(
                          po[:], v_sb[:, kk, h * HD:(h + 1) * HD],
                          aw[:, j * 512:(j + 1) * 512],
                          start=(kk == 0), stop=(kk == NKK - 1))
                  # denominator: level-1 bf16 pair add (two key tiles)
                  p1 = dnm.tile([128, QCH], BF16, tag="p1")
                  nc.vector.tensor_tensor(
                      p1[:], aw[:, 0:512], aw[:, 512:1024], op=OP.add)
                  pair.append(p1)
                  if len(pair) == 2:
                      a, b = pair[0], pair[1]
                      pair.clear()
                      if st_u["first"]:
                          st_u["first"] = False
                          nc.vector.tensor_tensor(dacc[:], a[:], b[:], op=OP.add)
                      else:
                          p2 = dnm.tile([128, QCH], BF16, tag="p2")
                          nc.vector.tensor_tensor(p2[:], a[:], b[:], op=OP.add)
                          nc.vector.tensor_tensor(dacc[:], dacc[:], p2[:], op=OP.add)
                  if kw == NKK // 2 - 1:
                      tail_pend.append(make_tail(qc, h, po, dacc))
                      if h == H_LOC - 1:
                          proj_pend.extend(make_proj(qc))

              aw_q = []
              DEFER = 3
              for u, (qc, h) in enumerate(units):
                  for kw in range(NKK // 2):
                      ps = psA.tile([128, 1024], F32, tag="s")
                      for j in range(2):
                          kk = 2 * kw + j
                          nc.tensor.matmul(
                              ps[:, j * 512:(j + 1) * 512],
                              kT[:, h, kk * 128:(kk + 1) * 128],
                              qT[:, h, qc * QCH:(qc + 1) * QCH],
                              start=True, stop=True)
                      aw = awp.tile([128, 1024], BF16, tag="aw")
                      nc.scalar.activation(aw[:], ps[:], AF.Exp, scale=SQRT_HD)
                      aw_q.append((u, kw, aw))
                      if len(aw_q) > DEFER:
                          if kw >= 6:
                              emit_tail_unit()
                          av_group(*aw_q.pop(0))
                      if kw >= 7 and kw % 2 == 1:
                          emit_proj_unit(2 if kw >= 11 else 1)
              while aw_q:
                  av_group(*aw_q.pop(0))
              emit_tail_unit()
              _flush_out = out_pend

              while proj_pend:
                  emit_proj_unit()
              while out_pend:
                  out_pend.pop(0)()

    nc.compile()
    return nc


def _get_nc():
    if "nc" not in _CACHE:
        _CACHE["nc"] = _build()
    return _CACHE["nc"]


def make_in_maps(x, freqs_cos, freqs_sin, wq, wk, wv, wo, sqk):
    bf16 = ml_dtypes.bfloat16
    xT = np.ascontiguousarray(
        np.asarray(x, np.float32).reshape(S, D).T).astype(bf16)
    cosk = np.ascontiguousarray(np.asarray(freqs_cos, np.float32)).astype(bf16)
    sink = np.ascontiguousarray(np.asarray(freqs_sin, np.float32)).astype(bf16)
    wq = np.asarray(wq, np.float32)
    wk = np.asarray(wk, np.float32)
    wv = np.asarray(wv, np.float32)
    wo = np.asarray(wo, np.float32)
    sqk_eff = (np.asarray(sqk, np.float32) * SQK_SCALE).astype(np.float32)
    in_maps = []
    for i in range(N_CORES):
        cols = slice(i * DH_LOC, (i + 1) * DH_LOC)
        wqkv = np.concatenate([wq[:, cols], wk[:, cols], wv[:, cols]], axis=1)
        in_maps.append({
            "xT": xT,
            "wqkv": np.ascontiguousarray(wqkv).astype(bf16),
            "wo": np.ascontiguousarray(wo[cols, :]).astype(bf16),
            "sqk": np.ascontiguousarray(
                np.concatenate([sqk_eff[cols]] * 2)),
            "cos2": cosk,
            "sin2": sink,
        })
    return in_maps


def assemble(results):
    y = np.empty((S, D), np.float32)
    w = RS_CHUNK // N_CORES
    w7 = RS_CHUNK // 2 // N_CORES
    for i in range(N_CORES):
        o = np.asarray(results[i]["out"]).astype(np.float32)
        for g in range(N_RS - 1):
            y[g * RS_CHUNK + i * w:g * RS_CHUNK + (i + 1) * w, :] = o[g]
        g = N_RS - 1
        base = g * RS_CHUNK
        for sub in range(2):
            r0 = base + sub * (RS_CHUNK // 2) + i * w7
            y[r0:r0 + w7, :] = o[g][sub * w7:(sub + 1) * w7]
    return y.reshape(1, S, D)


def kernel(**inputs):
    from concourse import bass_utils

    nc = _get_nc()
    in_maps = make_in_maps(**inputs)
    res = bass_utils.run_bass_kernel_spmd(nc, in_maps, core_ids=list(range(N_CORES)))
    return assemble(res.results)
